# revision 24
# baseline (speedup 1.0000x reference)
"""Trainium2 Bass kernel for per-batch spatial self-attention.

Per-core computation (one batch image per NeuronCore, 8 cores):
  x:(256, 4096)  ->  q/k = W x + b (channels-major),  v = x^T W^T + b (pixels-major)
  St[m,n] = sum_c K[c,m] Q[c,n]      (scores, TRANSPOSED: keys on partitions)
  Pt = exp(St/16)                     (softmax numerator, already transposed)
  OT[o,n] = sum_m V[m,o] Pt[m,n]     (attn @ v, channels-major output)
  out = OT / rowsum                   (rowsum via ones-matmul over partitions)

All matmuls run in float32r (tf32-like, 1 cycle/row at N>=256).
"""

import sys

sys.path.insert(0, "/opt/trn_rl_repo")

import numpy as np
import concourse.bacc as bacc
import concourse.mybir as mybir
import concourse.tile as tile
from concourse.bass_utils import run_bass_kernel_spmd

F32 = mybir.dt.float32
F32R = mybir.dt.float32r
AF = mybir.ActivationFunctionType

B = 8
C = 256  # channels
NPIX = 4096  # 64*64
NT = NPIX // 512  # 8 column tiles of 512 queries
M = NPIX // 128  # 32 key chunks of 128
SCALE = 1.0 / 16.0  # 1/sqrt(C)

_CACHE = {}


def _build():
    nc = bacc.Bacc("TRN2", num_swdge_queues=4)
    x_d = nc.declare_dram_parameter("x", [C, NPIX], F32, isOutput=False)
    wq_d = nc.declare_dram_parameter("wq_t", [C, C], F32, isOutput=False)
    wk_d = nc.declare_dram_parameter("wk_t", [C, C], F32, isOutput=False)
    wv_d = nc.declare_dram_parameter("wv_t", [C, C], F32, isOutput=False)
    bq_d = nc.declare_dram_parameter("bq", [C, 1], F32, isOutput=False)
    bk_d = nc.declare_dram_parameter("bk", [C, 1], F32, isOutput=False)
    bv_d = nc.declare_dram_parameter("bv", [1, C], F32, isOutput=False)
    out_d = nc.declare_dram_parameter("out", [C, NPIX], F32, isOutput=True)

    with tile.TileContext(nc) as tc:
        with (
            tc.tile_pool(name="big", bufs=1) as big,
            tc.tile_pool(name="small", bufs=2) as small,
            tc.tile_pool(name="ptp", bufs=8) as ptp,
            tc.tile_pool(name="outp", bufs=4) as outp,
            tc.tile_pool(name="psA", bufs=3, space="PSUM") as psA,
            tc.tile_pool(name="psB", bufs=4, space="PSUM") as psB,
            tc.tile_pool(name="psR", bufs=1, space="PSUM") as psR,
        ):
            # ---- load inputs (gpsimd DMA casts f32 -> f32r), need-ordered ----
            import concourse.bass as bass

            w_r = {}
            for nm, wd in (("q", wq_d), ("k", wk_d), ("v", wv_d)):
                w_r[nm] = [
                    big.tile([128, C], F32R, name=f"w{nm}_r{i}") for i in range(2)
                ]
            x_r = [big.tile([128, NPIX], F32R, name=f"x_r{i}") for i in range(2)]

            # fast path for the first-needed slices: raw HWDGE DMA + DVE round
            with tc.tile_pool(name="fraw", bufs=4) as fraw:
                for i in range(2):
                    rw = fraw.tile([128, C], F32, name="rw", tag="fw")
                    nc.sync.dma_start(out=rw, in_=wq_d[i * 128 : (i + 1) * 128, :])
                    nc.vector.tensor_copy(w_r["q"][i], rw)
                for i in range(2):
                    rx = fraw.tile([128, 512], F32, name="rx", tag="fx")
                    nc.sync.dma_start(out=rx, in_=x_d[i * 128 : (i + 1) * 128, 0:512])
                    nc.vector.tensor_copy(x_r[i][:, 0:512], rx)
                for i in range(2):
                    rw = fraw.tile([128, C], F32, name="rw2", tag="fw")
                    nc.sync.dma_start(out=rw, in_=wk_d[i * 128 : (i + 1) * 128, :])
                    nc.vector.tensor_copy(w_r["k"][i], rw)
                for i in range(2):
                    rx = fraw.tile([128, 512], F32, name="rx2", tag="fx")
                    nc.sync.dma_start(
                        out=rx, in_=x_d[i * 128 : (i + 1) * 128, 512:1024]
                    )
                    nc.vector.tensor_copy(x_r[i][:, 512:1024], rx)
            for i in range(2):
                nc.gpsimd.dma_start(
                    out=w_r["v"][i], in_=wv_d[i * 128 : (i + 1) * 128, :]
                )
            for j in range(2, 8):
                lo, hi = j * 512, (j + 1) * 512
                for i in range(2):
                    nc.gpsimd.dma_start(
                        out=x_r[i][:, lo:hi], in_=x_d[i * 128 : (i + 1) * 128, lo:hi]
                    )
            bq_sb = [big.tile([128, 1], F32, name=f"bq_sb{i}") for i in range(2)]
            for i in range(2):
                nc.sync.dma_start(out=bq_sb[i], in_=bq_d[i * 128 : (i + 1) * 128, :])

            bv_bc = big.tile([128, C], F32, name="bv_bc")
            bv_bcast_ap = bass.AP(
                tensor=bv_d.ap().tensor,
                offset=0,
                ap=[[0, 128], [1, C]],
            )
            nc.sync.dma_start(out=bv_bc, in_=bv_bcast_ap)

            ones_f = big.tile([128, 1], F32, name="ones_f")
            nc.vector.memset(ones_f, 1.0)
            ones_rf = big.tile([1, 128], F32, name="ones_rf")
            nc.vector.memset(ones_rf, 1.0)
            ones_col = big.tile([128, 1], F32R, name="ones_col")
            nc.vector.tensor_copy(ones_col, ones_f)
            ones_row = big.tile([1, 128], F32R, name="ones_row")
            nc.vector.tensor_copy(ones_row, ones_rf)

            # keep the PE busy (and the HAM clock-gate warm) while the first
            # input slices stream in; results are discarded
            warm_f = small.tile([128, 256], F32, name="warm_f", tag="warm_f")
            nc.vector.memset(warm_f, 1.0)
            warm_r = small.tile([128, 256], F32R, name="warm_r", tag="warm_r")
            nc.vector.tensor_copy(warm_r, warm_f)
            warm_ps = psR.tile([1, 256], F32, name="warm_ps", tag="psR")
            for _ in range(48):
                nc.tensor.matmul(
                    warm_ps, ones_col, warm_r, start=True, stop=True,
                    skip_group_check=True,
                )



            # ---- Q, K projections (channels-major: [o, pix]) ----
            q_sb = [big.tile([128, NPIX], F32R, name=f"q_sb{i}") for i in range(2)]
            k_sb = [big.tile([128, NPIX], F32R, name=f"k_sb{i}") for i in range(2)]
            for tgt, wkey, bias in ((q_sb, "q", bq_sb), (k_sb, "k", None)):
                for o in range(2):
                    for nt in range(NT):
                        ps = psA.tile([128, 512], F32, name="ps_proj", tag="psA")
                        for i in range(2):
                            nc.tensor.matmul(
                                ps,
                                w_r[wkey][i][:, o * 128 : (o + 1) * 128],
                                x_r[i][:, nt * 512 : (nt + 1) * 512],
                                start=(i == 0),
                                stop=(i == 1),
                            )
                        if bias is not None:
                            nc.vector.tensor_scalar_add(
                                tgt[o][:, nt * 512 : (nt + 1) * 512], ps, bias[o]
                            )
                        else:
                            # k bias is softmax-invariant (constant along keys
                            # for a fixed query); skip it
                            nc.scalar.activation(
                                tgt[o][:, nt * 512 : (nt + 1) * 512], ps, AF.Copy
                            )

            # ---- V projection (pixels-major: [pix, o]) ----
            v_sb = big.tile([128, M * C], F32R, name="v_sb")
            for m in range(M):
                ps = psA.tile([128, C], F32, name="ps_v", tag="psA")
                for i in range(2):
                    nc.tensor.matmul(
                        ps,
                        x_r[i][:, m * 128 : (m + 1) * 128],
                        w_r["v"][i],
                        start=(i == 0),
                        stop=(i == 1),
                    )
                nc.vector.tensor_add(v_sb[:, m * C : (m + 1) * C], ps, bv_bc)

            # ---- attention, one 512-query tile at a time ----
            for nt in range(NT):
                ot0 = psB.tile([128, 512], F32, name="ot0", tag="psB")
                ot1 = psB.tile([128, 512], F32, name="ot1", tag="psB")
                acc = small.tile([128, 512], F32R, name="acc", tag="acc")
                LAG = 4
                pts = {}
                for mm in range(M + LAG):
                    if mm < M:
                        m = mm
                        st = psA.tile([128, 512], F32, name="st", tag="psA")
                        for i in range(2):
                            nc.tensor.matmul(
                                st,
                                k_sb[i][:, m * 128 : (m + 1) * 128],
                                q_sb[i][:, nt * 512 : (nt + 1) * 512],
                                start=(i == 0),
                                stop=(i == 1),
                            )
                        pt = ptp.tile([128, 512], F32R, name="pt")
                        nc.scalar.activation(pt, st, AF.Exp, scale=SCALE)
                        pts[m] = pt
                    if mm >= LAG:
                        m = mm - LAG
                        pt = pts.pop(m)
                        if m == 0:
                            nc.vector.tensor_copy(acc, pt)
                        else:
                            nc.vector.tensor_add(acc, acc, pt)
                        nc.tensor.matmul(
                            ot0,
                            v_sb[:, m * C : m * C + 128],
                            pt,
                            start=(m == 0),
                            stop=(m == M - 1),
                        )
                        nc.tensor.matmul(
                            ot1,
                            v_sb[:, m * C + 128 : (m + 1) * C],
                            pt,
                            start=(m == 0),
                            stop=(m == M - 1),
                        )
                # rowsum over partitions -> reciprocal -> broadcast to 128 rows
                rs = psR.tile([1, 512], F32, name="rs", tag="psR")
                nc.tensor.matmul(rs, ones_col, acc, start=True, stop=True)
                rinv_f = small.tile([1, 512], F32, name="rinv_f", tag="rinv_f")
                rinv_s = small.tile([1, 512], F32, name="rinv_s", tag="rinv_s")
                nc.vector.reciprocal_approx_accurate(rinv_f, rs, rinv_s)
                rinv = small.tile([1, 512], F32R, name="rinv", tag="rinv")
                nc.vector.tensor_copy(rinv, rinv_f)
                rb = psR.tile([128, 512], F32, name="rb", tag="psR")
                nc.tensor.matmul(rb, ones_row, rinv, start=True, stop=True)
                rb_sb = small.tile([128, 512], F32, name="rb_sb", tag="rb_sb")
                nc.vector.tensor_copy(rb_sb, rb)
                for o, ot in enumerate((ot0, ot1)):
                    osb = outp.tile([128, 512], F32, name="osb", tag="osb")
                    nc.vector.tensor_mul(osb, ot, rb_sb)
                    nc.sync.dma_start(
                        out=out_d[o * 128 : (o + 1) * 128, nt * 512 : (nt + 1) * 512],
                        in_=osb,
                    )

    nc.compile()
    return nc


def _get_nc():
    if "nc" not in _CACHE:
        _CACHE["nc"] = _build()
    return _CACHE["nc"]


def kernel(x, wq, wk, wv, bq, bk, bv):
    x = np.asarray(x, dtype=np.float32)
    wq = np.asarray(wq, dtype=np.float32)
    wk = np.asarray(wk, dtype=np.float32)
    wv = np.asarray(wv, dtype=np.float32)
    bq = np.asarray(bq, dtype=np.float32)
    bk = np.asarray(bk, dtype=np.float32)
    bv = np.asarray(bv, dtype=np.float32)

    nc = _get_nc()
    shared = {
        "wq_t": np.ascontiguousarray(wq.T),
        "wk_t": np.ascontiguousarray(wk.T),
        "wv_t": np.ascontiguousarray(wv.T),
        "bq": np.ascontiguousarray(bq.reshape(C, 1)),
        "bk": np.ascontiguousarray(bk.reshape(C, 1)),
        "bv": np.ascontiguousarray(bv.reshape(1, C)),
    }
    in_maps = [
        {"x": np.ascontiguousarray(x[b].reshape(C, NPIX)), **shared} for b in range(B)
    ]
    res = run_bass_kernel_spmd(nc, in_maps, core_ids=list(range(B)))
    out = np.stack([res.results[b]["out"] for b in range(B)])
    return out.reshape(B, C, 64, 64)


# revision 25
# speedup vs baseline: 1.0069x; 1.0069x over previous
"""Trainium2 Bass kernel for per-batch spatial self-attention.

Per-core computation (one batch image per NeuronCore, 8 cores):
  x:(256, 4096)  ->  q/k = W x + b (channels-major),  v = x^T W^T + b (pixels-major)
  St[m,n] = sum_c K[c,m] Q[c,n]      (scores, TRANSPOSED: keys on partitions)
  Pt = exp(St/16)                     (softmax numerator, already transposed)
  OT[o,n] = sum_m V[m,o] Pt[m,n]     (attn @ v, channels-major output)
  out = OT / rowsum                   (rowsum via ones-matmul over partitions)

All matmuls run in float32r (tf32-like, 1 cycle/row at N>=256).
"""

import sys

sys.path.insert(0, "/opt/trn_rl_repo")

import numpy as np
import concourse.bacc as bacc
import concourse.mybir as mybir
import concourse.tile as tile
from concourse.bass_utils import run_bass_kernel_spmd

F32 = mybir.dt.float32
F32R = mybir.dt.float32r
AF = mybir.ActivationFunctionType

B = 8
C = 256  # channels
NPIX = 4096  # 64*64
NT = NPIX // 512  # 8 column tiles of 512 queries
M = NPIX // 128  # 32 key chunks of 128
SCALE = 1.0 / 16.0  # 1/sqrt(C)

_CACHE = {}


def _build():
    nc = bacc.Bacc("TRN2", num_swdge_queues=4)
    x_d = nc.declare_dram_parameter("x", [C, NPIX], F32, isOutput=False)
    wq_d = nc.declare_dram_parameter("wq_t", [C, C], F32, isOutput=False)
    wk_d = nc.declare_dram_parameter("wk_t", [C, C], F32, isOutput=False)
    wv_d = nc.declare_dram_parameter("wv_t", [C, C], F32, isOutput=False)
    bq_d = nc.declare_dram_parameter("bq", [C, 1], F32, isOutput=False)
    bk_d = nc.declare_dram_parameter("bk", [C, 1], F32, isOutput=False)
    bv_d = nc.declare_dram_parameter("bv", [1, C], F32, isOutput=False)
    out_d = nc.declare_dram_parameter("out", [C, NPIX], F32, isOutput=True)

    with tile.TileContext(nc) as tc:
        with (
            tc.tile_pool(name="big", bufs=1) as big,
            tc.tile_pool(name="small", bufs=2) as small,
            tc.tile_pool(name="ptp", bufs=8) as ptp,
            tc.tile_pool(name="outp", bufs=4) as outp,
            tc.tile_pool(name="psA", bufs=3, space="PSUM") as psA,
            tc.tile_pool(name="psB", bufs=4, space="PSUM") as psB,
            tc.tile_pool(name="psR", bufs=1, space="PSUM") as psR,
        ):
            # ---- load inputs (gpsimd DMA casts f32 -> f32r), need-ordered ----
            import concourse.bass as bass

            w_r = {}
            for nm, wd in (("q", wq_d), ("k", wk_d), ("v", wv_d)):
                w_r[nm] = [
                    big.tile([128, C], F32R, name=f"w{nm}_r{i}") for i in range(2)
                ]
            x_r = [big.tile([128, NPIX], F32R, name=f"x_r{i}") for i in range(2)]

            # fast path for the first-needed slices: raw HWDGE DMA + DVE round
            with tc.tile_pool(name="fraw", bufs=4) as fraw:
                for i in range(2):
                    rw = fraw.tile([128, C], F32, name="rw", tag="fw")
                    nc.sync.dma_start(out=rw, in_=wq_d[i * 128 : (i + 1) * 128, :])
                    nc.vector.tensor_copy(w_r["q"][i], rw)
                for i in range(2):
                    rx = fraw.tile([128, 512], F32, name="rx", tag="fx")
                    nc.sync.dma_start(out=rx, in_=x_d[i * 128 : (i + 1) * 128, 0:512])
                    nc.vector.tensor_copy(x_r[i][:, 0:512], rx)
                for i in range(2):
                    rw = fraw.tile([128, C], F32, name="rw2", tag="fw")
                    nc.sync.dma_start(out=rw, in_=wk_d[i * 128 : (i + 1) * 128, :])
                    nc.vector.tensor_copy(w_r["k"][i], rw)
                for i in range(2):
                    rx = fraw.tile([128, 512], F32, name="rx2", tag="fx")
                    nc.sync.dma_start(
                        out=rx, in_=x_d[i * 128 : (i + 1) * 128, 512:1024]
                    )
                    nc.vector.tensor_copy(x_r[i][:, 512:1024], rx)
            for i in range(2):
                nc.gpsimd.dma_start(
                    out=w_r["v"][i], in_=wv_d[i * 128 : (i + 1) * 128, :]
                )
            for j in range(2, 8):
                lo, hi = j * 512, (j + 1) * 512
                for i in range(2):
                    nc.gpsimd.dma_start(
                        out=x_r[i][:, lo:hi], in_=x_d[i * 128 : (i + 1) * 128, lo:hi]
                    )
            bq_sb = [big.tile([128, 1], F32, name=f"bq_sb{i}") for i in range(2)]
            for i in range(2):
                nc.sync.dma_start(out=bq_sb[i], in_=bq_d[i * 128 : (i + 1) * 128, :])

            bv_bc = big.tile([128, C], F32, name="bv_bc")
            bv_bcast_ap = bass.AP(
                tensor=bv_d.ap().tensor,
                offset=0,
                ap=[[0, 128], [1, C]],
            )
            nc.sync.dma_start(out=bv_bc, in_=bv_bcast_ap)

            ones_f = big.tile([128, 1], F32, name="ones_f")
            nc.vector.memset(ones_f, 1.0)
            ones_rf = big.tile([1, 128], F32, name="ones_rf")
            nc.vector.memset(ones_rf, 1.0)
            ones_col = big.tile([128, 1], F32R, name="ones_col")
            nc.vector.tensor_copy(ones_col, ones_f)
            ones_row = big.tile([1, 128], F32R, name="ones_row")
            nc.vector.tensor_copy(ones_row, ones_rf)

            # keep the PE busy (and the HAM clock-gate warm) while the first
            # input slices stream in; results are discarded
            warm_f = small.tile([128, 256], F32, name="warm_f", tag="warm_f")
            nc.vector.memset(warm_f, 1.0)
            warm_r = small.tile([128, 256], F32R, name="warm_r", tag="warm_r")
            nc.vector.tensor_copy(warm_r, warm_f)
            warm_ps = psR.tile([1, 256], F32, name="warm_ps", tag="psR")
            for _ in range(48):
                nc.tensor.matmul(
                    warm_ps, ones_col, warm_r, start=True, stop=True,
                    skip_group_check=True,
                )



            # ---- Q, K projections (channels-major: [o, pix]) ----
            q_sb = [big.tile([128, NPIX], F32R, name=f"q_sb{i}") for i in range(2)]
            k_sb = [big.tile([128, NPIX], F32R, name=f"k_sb{i}") for i in range(2)]
            for tgt, wkey, bias in ((q_sb, "q", bq_sb), (k_sb, "k", None)):
                for o in range(2):
                    for nt in range(NT):
                        ps = psA.tile([128, 512], F32, name="ps_proj", tag="psA")
                        for i in range(2):
                            nc.tensor.matmul(
                                ps,
                                w_r[wkey][i][:, o * 128 : (o + 1) * 128],
                                x_r[i][:, nt * 512 : (nt + 1) * 512],
                                start=(i == 0),
                                stop=(i == 1),
                            )
                        if bias is not None:
                            nc.vector.tensor_scalar_add(
                                tgt[o][:, nt * 512 : (nt + 1) * 512], ps, bias[o]
                            )
                        else:
                            # k bias is softmax-invariant (constant along keys
                            # for a fixed query); skip it
                            nc.scalar.activation(
                                tgt[o][:, nt * 512 : (nt + 1) * 512], ps, AF.Copy
                            )

            # ---- V projection (pixels-major: [pix, o]) ----
            v_sb = big.tile([128, M * C], F32R, name="v_sb")
            for m in range(M):
                ps = psA.tile([128, C], F32, name="ps_v", tag="psA")
                for i in range(2):
                    nc.tensor.matmul(
                        ps,
                        x_r[i][:, m * 128 : (m + 1) * 128],
                        w_r["v"][i],
                        start=(i == 0),
                        stop=(i == 1),
                    )
                nc.vector.tensor_add(v_sb[:, m * C : (m + 1) * C], ps, bv_bc)

            # ---- attention, one 512-query tile at a time ----
            for nt in range(NT):
                ot0 = psB.tile([128, 512], F32, name="ot0", tag="psB")
                ot1 = psB.tile([128, 512], F32, name="ot1", tag="psB")
                acc = small.tile([128, 512], F32R, name="acc", tag="acc")
                LAG = 6
                pts = {}
                for mm in range(M + LAG):
                    if mm < M:
                        m = mm
                        st = psA.tile([128, 512], F32, name="st", tag="psA")
                        for i in range(2):
                            nc.tensor.matmul(
                                st,
                                k_sb[i][:, m * 128 : (m + 1) * 128],
                                q_sb[i][:, nt * 512 : (nt + 1) * 512],
                                start=(i == 0),
                                stop=(i == 1),
                            )
                        pt = ptp.tile([128, 512], F32R, name="pt")
                        nc.scalar.activation(pt, st, AF.Exp, scale=SCALE)
                        pts[m] = pt
                    if mm >= LAG:
                        m = mm - LAG
                        pt = pts.pop(m)
                        if m == 0:
                            nc.vector.tensor_copy(acc, pt)
                        else:
                            nc.vector.tensor_add(acc, acc, pt)
                        nc.tensor.matmul(
                            ot0,
                            v_sb[:, m * C : m * C + 128],
                            pt,
                            start=(m == 0),
                            stop=(m == M - 1),
                        )
                        nc.tensor.matmul(
                            ot1,
                            v_sb[:, m * C + 128 : (m + 1) * C],
                            pt,
                            start=(m == 0),
                            stop=(m == M - 1),
                        )
                # rowsum over partitions -> reciprocal -> broadcast to 128 rows
                rs = psR.tile([1, 512], F32, name="rs", tag="psR")
                nc.tensor.matmul(rs, ones_col, acc, start=True, stop=True)
                rinv_f = small.tile([1, 512], F32, name="rinv_f", tag="rinv_f")
                rinv_s = small.tile([1, 512], F32, name="rinv_s", tag="rinv_s")
                nc.vector.reciprocal_approx_accurate(rinv_f, rs, rinv_s)
                rinv = small.tile([1, 512], F32R, name="rinv", tag="rinv")
                nc.vector.tensor_copy(rinv, rinv_f)
                rb = psR.tile([128, 512], F32, name="rb", tag="psR")
                nc.tensor.matmul(rb, ones_row, rinv, start=True, stop=True)
                rb_sb = small.tile([128, 512], F32, name="rb_sb", tag="rb_sb")
                nc.vector.tensor_copy(rb_sb, rb)
                for o, ot in enumerate((ot0, ot1)):
                    osb = outp.tile([128, 512], F32, name="osb", tag="osb")
                    nc.vector.tensor_mul(osb, ot, rb_sb)
                    nc.sync.dma_start(
                        out=out_d[o * 128 : (o + 1) * 128, nt * 512 : (nt + 1) * 512],
                        in_=osb,
                    )

    nc.compile()
    return nc


def _get_nc():
    if "nc" not in _CACHE:
        _CACHE["nc"] = _build()
    return _CACHE["nc"]


def kernel(x, wq, wk, wv, bq, bk, bv):
    x = np.asarray(x, dtype=np.float32)
    wq = np.asarray(wq, dtype=np.float32)
    wk = np.asarray(wk, dtype=np.float32)
    wv = np.asarray(wv, dtype=np.float32)
    bq = np.asarray(bq, dtype=np.float32)
    bk = np.asarray(bk, dtype=np.float32)
    bv = np.asarray(bv, dtype=np.float32)

    nc = _get_nc()
    shared = {
        "wq_t": np.ascontiguousarray(wq.T),
        "wk_t": np.ascontiguousarray(wk.T),
        "wv_t": np.ascontiguousarray(wv.T),
        "bq": np.ascontiguousarray(bq.reshape(C, 1)),
        "bk": np.ascontiguousarray(bk.reshape(C, 1)),
        "bv": np.ascontiguousarray(bv.reshape(1, C)),
    }
    in_maps = [
        {"x": np.ascontiguousarray(x[b].reshape(C, NPIX)), **shared} for b in range(B)
    ]
    res = run_bass_kernel_spmd(nc, in_maps, core_ids=list(range(B)))
    out = np.stack([res.results[b]["out"] for b in range(B)])
    return out.reshape(B, C, 64, 64)
